# revision 27
# baseline (speedup 1.0000x reference)
"""Trainium2 Bass kernel: per-sample mean-pool over valid tokens + 4x head repeat.

Problem: encoded_batch [32, 2048, 1024] f32 with padding rows exactly zero,
text_lengths [32]. Output [32, 4096] = repeat(mean over valid tokens, 4).

Host-side prep (kernel() is a host function; packing is layout prep, the
reduction itself runs on device): samples are bin-packed 4-per-core and each
core's valid rows are packed into TWO contiguous streams:
  - fp8e4m3 for long samples (len >= 1024): elementwise rel err ~2^-4
    averages down by sqrt(n) over >=1024 rows -> ~5e-3 final rel err.
  - bf16 for short samples: rel err ~2^-9, fine at any length.
Raw values are packed (no pre-scaling, which would hit fp8's subnormal
floor); the 1/len scale is applied once to the f32 PSUM result. Streaming
8/16-bit instead of f32 cuts HBM traffic ~3.4x for this memory-bound
reduction. All cores stream the same padded block counts (SPMD program
depends only on (T8, T16)).

On device a single SPMD program accumulates all four samples into one
[4, 1024] f32 PSUM tile via selector matmuls: sel[:, 4t+m] = 1 iff block t's
partition row belongs to sample slot m (data-driven routing -> correct for
arbitrary inputs). The fp8 region streams first so the tensor engine
(matmul-rate-bound on fp8) catches back up during the bf16 region, whose
tiles taper to 128 rows at the end. Epilogue: one DVE multiply by 1/len
(PSUM -> SBUF) and one 16 KB output DMA; the 4x head repeat is pure layout
and happens in the host-side gather.

Sharding: pure data parallel across 8 NeuronCores, no cross-core traffic.
"""

import numpy as np
import ml_dtypes

import concourse.bass as bass
import concourse.bass_utils as _bass_utils
import concourse.tile as tile
from concourse import bacc, mybir
from concourse.bass_utils import run_bass_kernel_spmd

# Cap the compiler's semaphore allocation: the NEFF wrapper's teardown
# clears every allocated semaphore one instruction at a time (~23ns each,
# serialized on the sem file), so the default 256-sem claim costs ~6us of
# pure epilogue on every run. This kernel uses ~40.
MAX_SEM_NUM = 96
if not getattr(_bass_utils, "_semcap_patched", False):
    _orig_get_walrus_args = _bass_utils.get_walrus_args

    def _get_walrus_args(*a, **k):
        return _orig_get_walrus_args(*a, **k) + [f"--max-sem-num={MAX_SEM_NUM}"]

    _bass_utils.get_walrus_args = _get_walrus_args
    _bass_utils._semcap_patched = True

B, S, D = 32, 2048, 1024
NH = 4
N_CORES = 8
BPC = B // N_CORES            # sample slots per core
P = 128
THRESH = 512                  # len >= THRESH -> fp8 stream

BF16 = ml_dtypes.bfloat16
FP8 = ml_dtypes.float8_e4m3   # matches mybir.dt.float8e4

_CACHE = {}
LAST_RESULTS = None  # BassKernelResults of the most recent kernel() call


def _split8(rows):
    """fp8 region DMA tile row counts (multiples of 256 for DoubleRow):
    ramp up so the first matmuls start early, 1024-row tiles in the middle,
    taper down so matmuls finish right behind the last bytes."""
    out = []
    rem = rows
    for sz in (256, 512):
        if rem >= sz + 1792:
            out.append(sz)
            rem -= sz
    while rem > 1792:
        out.append(1024)
        rem -= 1024
    for sz in (768, 512, 256):
        while rem >= sz:
            out.append(sz)
            rem -= sz
    assert rem == 0
    return out


def _split16(rows):
    """bf16 region DMA tile row counts, tapering to 128 at the end."""
    out = []
    rem = rows
    while rem > 1664:
        out.append(1024)
        rem -= 1024
    while rem > 768:
        out.append(512)
        rem -= 512
    while rem > 256:
        out.append(256)
        rem -= 256
    while rem > 0:
        out.append(128)
        rem -= 128
    return out


def _build(T8, T16):
    """Build the SPMD program for T8 fp8 + T16 bf16 128-row blocks."""
    f32 = mybir.dt.float32
    bf16 = mybir.dt.bfloat16
    fp8 = mybir.dt.float8e4
    nc = bacc.Bacc("TRN2", target_bir_lowering=False, debug=False)
    # Drop DMA queues this kernel never uses (gpsimd SWDGE + ACT HWDGE ring):
    # fewer queues for the NEFF wrapper to initialize and tear down.
    nc.m.queues = [q for q in nc.m.queues if q.name == "qSPDynamicHW"]

    assert T8 % 2 == 0
    T2 = T8 // 2  # fp8 super-groups (256 rows each, DoubleRow matmuls)
    # DoubleRow LDWEIGHTS requires the Ko=2 dim's step to be a multiple of
    # 16 elements (s3_lw dual-fp8 restriction), so the per-(t, j) selector
    # row is padded from NH=4 to SEL_PAD entries.
    SEL_PAD = 16
    if T8:
        x8 = nc.declare_dram_parameter("x8", [T8 * P, D], fp8, isOutput=False)
        sel8 = nc.declare_dram_parameter(
            "sel8", [P, T2 * 2 * SEL_PAD], fp8, isOutput=False
        )
    if T16:
        x16 = nc.declare_dram_parameter("x16", [T16 * P, D], bf16, isOutput=False)
        sel16 = nc.declare_dram_parameter("sel16", [P, NH * T16], bf16, isOutput=False)
    scale = nc.declare_dram_parameter(
        f"scale_semcap{MAX_SEM_NUM}", [BPC, 1], f32, isOutput=False
    )
    out = nc.declare_dram_parameter("out", [BPC, D], f32, isOutput=True)

    n_acc = T2 + T16  # accumulation steps (super-groups + plain groups)

    with tile.TileContext(nc) as tc:
        with (
            tc.tile_pool(name="xin", bufs=5) as xpool,
            tc.tile_pool(name="acc", bufs=1, space="PSUM") as psum_pool,
            tc.tile_pool(name="aux", bufs=1) as aux,
        ):
            # Dispatch the FIRST x tile before the tiny sel/scale loads so the
            # big stream starts moving ~1.5us earlier (dispatches are serial
            # on the Sync engine; the selector transfer itself is <0.2us).
            first_xt = None
            if T8:
                tiles8 = _split8(T8 * P)
                rows0 = tiles8[0]
                src0 = x8.ap()[0:rows0, :].rearrange("(p a) d -> p (a d)", p=P)
                first_xt = xpool.tile([P, rows0 // 256, 2, D], fp8, tag="xt8")
                nc.sync.dma_start(first_xt[:], src0)

            if T8:
                sel8_sb = aux.tile([P, T2, 2, SEL_PAD], fp8)
                nc.sync.dma_start(sel8_sb[:], sel8.ap())
            if T16:
                sel16_sb = aux.tile([P, NH * T16], bf16)
                nc.sync.dma_start(sel16_sb[:], sel16.ap())
            scale_sb = aux.tile([BPC, 1], f32)
            nc.sync.dma_start(scale_sb[:], scale.ap())

            # Pre-warm the ACT Copy table so LoadActFuncSet (~1.5us) runs
            # during the stream, not in the epilogue.
            warm = aux.tile([1, 1], f32)
            nc.scalar.activation(
                warm[:], scale_sb[0:1, 0:1],
                mybir.ActivationFunctionType.Copy, scale=1.0,
            )

            ps = psum_pool.tile([BPC, D], f32)
            a_idx = 0  # global accumulation step

            # fp8 region: DoubleRow matmuls contract 256 rows (2 k-subtiles)
            # per pass at 2 rows/cycle.
            if T8:
                tiles = tiles8
                assert sum(tiles) == T8 * P
                row_off = 0
                t2_idx = 0
                for ti, rows in enumerate(tiles):
                    g2 = rows // 256
                    src = x8.ap()[row_off : row_off + rows, :].rearrange(
                        "(p a) d -> p (a d)", p=P
                    )
                    row_off += rows
                    if ti == 0:
                        xt = first_xt
                    else:
                        xt = xpool.tile([P, g2, 2, D], fp8, tag="xt8")
                        nc.sync.dma_start(xt[:], src)
                    for g in range(g2):
                        w = sel8_sb[:, t2_idx, :, 0:NH]
                        for h in range(D // 512):
                            nc.tensor.matmul(
                                ps[0:BPC, h * 512 : (h + 1) * 512],
                                w,
                                xt[:, g, :, h * 512 : (h + 1) * 512],
                                start=(a_idx == 0),
                                stop=(a_idx == n_acc - 1),
                                perf_mode=mybir.MatmulPerfMode.DoubleRow,
                            )
                        t2_idx += 1
                        a_idx += 1
                assert t2_idx == T2

            # bf16 region: plain matmuls over 128-row groups, tapered tiles.
            if T16:
                tiles = _split16(T16 * P)
                assert sum(tiles) == T16 * P
                row_off = 0
                t_idx = 0
                for rows in tiles:
                    rpp = rows // P
                    src = x16.ap()[row_off : row_off + rows, :].rearrange(
                        "(p a) d -> p (a d)", p=P
                    )
                    row_off += rows
                    xt = xpool.tile([P, rpp * D], bf16, tag="xt16")
                    nc.sync.dma_start(xt[:], src)
                    for r in range(rpp):
                        w = sel16_sb[:, NH * t_idx : NH * (t_idx + 1)]
                        for h in range(D // 512):
                            c0 = r * D + h * 512
                            nc.tensor.matmul(
                                ps[0:BPC, h * 512 : (h + 1) * 512],
                                w,
                                xt[:, c0 : c0 + 512],
                                start=(a_idx == 0),
                                stop=(a_idx == n_acc - 1),
                            )
                        t_idx += 1
                        a_idx += 1
                assert t_idx == T16
            assert a_idx == n_acc

            # PSUM holds raw per-slot sums; scale by 1/len (per-partition
            # scalar) on the way to SBUF: DVE lower half, pre-warmed ACT
            # upper half in parallel, then one 16 KB output DMA.
            h2 = D // 2
            out_sb = aux.tile([BPC, D], f32)
            nc.vector.tensor_scalar_mul(
                out_sb[:, 0:h2], ps[0:BPC, 0:h2], scale_sb[:, 0:1]
            )
            nc.scalar.activation(
                out_sb[:, h2:D], ps[0:BPC, h2:D],
                mybir.ActivationFunctionType.Copy, scale=scale_sb[:, 0:1],
            )
            nc.sync.dma_start(out.ap()[:, :], out_sb[:])

    nc.compile()
    return nc


def _pack_bins(lengths):
    """Assign samples to cores (BPC each), minimizing the padded stream cost
    (T8 + 2*T16 blocks, then total groups, then max rows) via LPT seed +
    randomized swaps."""
    nrows = np.maximum(1, lengths).astype(np.int64)
    is8 = nrows >= THRESH

    def cost(bins_):
        r8 = [sum(int(nrows[i]) for i in b if is8[i]) for b in bins_]
        r16 = [sum(int(nrows[i]) for i in b if not is8[i]) for b in bins_]
        T8 = max(-(-r // P) for r in r8)
        T16 = max(-(-r // P) for r in r16)
        return (T8 + 2 * T16, T8 + T16, max(a + b for a, b in zip(r8, r16)))

    # LPT seed on raw rows
    bins = [[] for _ in range(N_CORES)]
    tot = [0] * N_CORES
    for i in np.argsort(-nrows, kind="stable"):
        c = min(
            (c for c in range(N_CORES) if len(bins[c]) < BPC),
            key=lambda c: (tot[c], len(bins[c])),
        )
        bins[c].append(int(i))
        tot[c] += int(nrows[i])

    best = cost(bins)
    rng = np.random.RandomState(0)
    for _ in range(4000):
        c1, c2 = rng.randint(0, N_CORES, 2)
        if c1 == c2:
            continue
        a, b = rng.randint(0, BPC, 2)
        bins[c1][a], bins[c2][b] = bins[c2][b], bins[c1][a]
        cand = cost(bins)
        if cand <= best:
            best = cand
        else:
            bins[c1][a], bins[c2][b] = bins[c2][b], bins[c1][a]
    return bins, best


def kernel(**inputs) -> np.ndarray:
    global LAST_RESULTS
    x = np.asarray(inputs["encoded_batch"])
    if x.dtype != np.float32:
        x = x.astype(np.float32)
    lengths = np.asarray(inputs["text_lengths"]).astype(np.int64)
    assert x.shape == (B, S, D), x.shape

    nrows = np.maximum(1, lengths).astype(np.int64)
    is8 = nrows >= THRESH
    bins, (cost0, _, _) = _pack_bins(lengths)
    r8 = [sum(int(nrows[i]) for i in b if is8[i]) for b in bins]
    r16 = [sum(int(nrows[i]) for i in b if not is8[i]) for b in bins]
    T8 = max(-(-r // P) for r in r8)
    T8 += T8 % 2  # DoubleRow super-groups need an even block count
    T16 = max(-(-r // P) for r in r16)

    if (T8, T16) not in _CACHE:
        _CACHE[(T8, T16)] = _build(T8, T16)
    nc = _CACHE[(T8, T16)]

    inv = (np.float64(1.0) / lengths.astype(np.float64)).astype(np.float32)
    pidx = np.arange(P)

    def pack_stream(samples, T, np_dt):
        xp = np.zeros((T * P, D), dtype=np_dt)
        row_slot = np.full(T * P, -1, dtype=np.int64)
        off = 0
        for m, i in samples:
            nr = int(nrows[i])
            xp[off : off + nr] = x[i, :nr].astype(np_dt)
            row_slot[off : off + nr] = m
            off += nr
        return xp, row_slot

    def sel_plain(row_slot, T, np_dt, tiles):
        """Selector for plain matmuls: partition p of group (tile, r) holds
        stream row base + p*rpp + r."""
        selc = np.zeros((P, NH * T), dtype=np_dt)
        t = 0
        base = 0
        for rows_ in tiles:
            rpp = rows_ // P
            for r in range(rpp):
                rs = row_slot[base + pidx * rpp + r]
                valid = rs >= 0
                selc[pidx[valid], NH * t + rs[valid]] = 1.0
                t += 1
            base += rows_
        assert t == T
        return selc

    def sel_double(row_slot, T2, np_dt, tiles):
        """Selector for DoubleRow matmuls: (partition p, subtile j) of
        super-group (tile, g) holds stream row base + p*q + 2g + j, where
        q = 2 * (tile rows / 256)."""
        selc = np.zeros((P, T2, 2, 16), dtype=np_dt)
        t = 0
        base = 0
        for rows_ in tiles:
            g2 = rows_ // 256
            q = 2 * g2
            for g in range(g2):
                for j in range(2):
                    rs = row_slot[base + pidx * q + 2 * g + j]
                    valid = rs >= 0
                    selc[pidx[valid], t, j, rs[valid]] = 1.0
                t += 1
            base += rows_
        assert t == T2
        return selc.reshape(P, T2 * 2 * 16)

    tiles8 = _split8(T8 * P) if T8 else []
    tiles16 = _split16(T16 * P) if T16 else []
    in_maps = []
    for c in range(N_CORES):
        m8 = [(m, i) for m, i in enumerate(bins[c]) if is8[i]]
        m16 = [(m, i) for m, i in enumerate(bins[c]) if not is8[i]]
        im = {f"scale_semcap{MAX_SEM_NUM}": inv[bins[c]].reshape(BPC, 1)}
        if T8:
            im["x8"], slot8 = pack_stream(m8, T8, FP8)
            im["sel8"] = sel_double(slot8, T8 // 2, FP8, tiles8)
        if T16:
            im["x16"], slot16 = pack_stream(m16, T16, BF16)
            im["sel16"] = sel_plain(slot16, T16, BF16, tiles16)
        in_maps.append(im)

    res = run_bass_kernel_spmd(nc, in_maps, list(range(N_CORES)))
    LAST_RESULTS = res

    full = np.empty((B, D * NH), dtype=np.float32)
    for c in range(N_CORES):
        mean_c = res.results[c]["out"]  # [BPC, D] f32
        full[bins[c]] = np.repeat(mean_c, NH, axis=-1)
    return full


# revision 31
# speedup vs baseline: 1.0887x; 1.0887x over previous
"""Trainium2 Bass kernel: per-sample mean-pool over valid tokens + 4x head repeat.

Problem: encoded_batch [32, 2048, 1024] f32 with padding rows exactly zero,
text_lengths [32]. Output [32, 4096] = repeat(mean over valid tokens, 4).

Host-side prep (kernel() is a host function; packing is layout prep, the
reduction itself runs on device): samples are bin-packed 4-per-core and each
core's valid rows are packed into TWO contiguous streams:
  - bf16 for short samples (len < 512): rel err ~2^-9, fine at any length.
  - fp8e4m3 for long samples (len >= 512): elementwise rel err ~2^-4
    averages down by sqrt(n) over the sequence -> <1e-2 final rel err.
Raw values are packed (no pre-scaling, which would hit fp8's subnormal
floor); the 1/len scale is applied once to the f32 PSUM result. Streaming
8/16-bit instead of f32 cuts HBM traffic ~3.6x for this memory-bound
reduction. All cores stream the same padded block counts (the SPMD program
depends only on (T16, T8, K2)).

On device a single SPMD program accumulates all four samples into one
[4, 1024] f32 PSUM tile via selector matmuls: sel[:, 4t+m] = 1 iff the row
at that (partition, subtile) position belongs to sample slot m (data-driven
routing -> correct for arbitrary inputs). The bf16 region (plain matmuls)
streams first; the fp8 region (DoubleRow matmuls, 2 k-subtiles of 128 rows
per pass at 2 rows/cycle) follows. Each core's LONGEST fp8 sample is pinned
to slot 0 and packed at the END of the stream, so the trailing K2
super-group tiles only contribute to PSUM row 0: rows 1-3 are scaled
(1/len, one DVE op) and DMA'd out while the stream is still running,
leaving just a [1, 1024] DVE copy and a 4 KB DMA after the last matmul.
The 4x head repeat is pure layout and happens in the host-side gather.

Sharding: pure data parallel across 8 NeuronCores, no cross-core traffic.
"""

import numpy as np
import ml_dtypes

import concourse.tile as tile
from concourse import bacc, mybir
from concourse.bass_utils import run_bass_kernel_spmd

B, S, D = 32, 2048, 1024
NH = 4
N_CORES = 8
BPC = B // N_CORES            # sample slots per core
P = 128
THRESH = 512                  # len >= THRESH -> fp8 stream

BF16 = ml_dtypes.bfloat16
FP8 = ml_dtypes.float8_e4m3   # matches mybir.dt.float8e4

_CACHE = {}
LAST_RESULTS = None  # BassKernelResults of the most recent kernel() call


def _split8(rows):
    """fp8 region DMA tile row counts (multiples of 256 for DoubleRow):
    ramp up so the first matmuls start early, 1024-row tiles in the middle,
    taper down so matmuls finish right behind the last bytes."""
    out = []
    rem = rows
    for sz in (256, 512):
        if rem >= sz + 1792:
            out.append(sz)
            rem -= sz
    while rem > 1792:
        out.append(1024)
        rem -= 1024
    for sz in (768, 512, 256):
        while rem >= sz:
            out.append(sz)
            rem -= sz
    assert rem == 0
    return out


def _split16(rows):
    """bf16 region DMA tile row counts."""
    out = []
    rem = rows
    while rem > 1664:
        out.append(1024)
        rem -= 1024
    while rem > 768:
        out.append(512)
        rem -= 512
    while rem > 256:
        out.append(256)
        rem -= 256
    while rem > 0:
        out.append(128)
        rem -= 128
    return out


def _build(T16, T8, K2):
    """SPMD program: T16 bf16 blocks, then T8 fp8 blocks whose trailing K2
    super-groups write only PSUM row 3 (pure-tail overlap)."""
    f32 = mybir.dt.float32
    bf16 = mybir.dt.bfloat16
    fp8 = mybir.dt.float8e4
    nc = bacc.Bacc("TRN2", target_bir_lowering=False, debug=False)
    # Drop DMA queues this kernel never uses (gpsimd SWDGE + ACT HWDGE ring).
    nc.m.queues = [q for q in nc.m.queues if q.name == "qSPDynamicHW"]

    assert T8 % 2 == 0
    T2 = T8 // 2
    assert 0 <= K2 <= T2
    # DoubleRow LDWEIGHTS requires the Ko=2 dim's step to be a multiple of
    # 16 elements (s3_lw dual-fp8 restriction) -> selector padded to 16.
    SEL_PAD = 16
    if T16:
        x16 = nc.declare_dram_parameter("x16", [T16 * P, D], bf16, isOutput=False)
        sel16 = nc.declare_dram_parameter("sel16", [P, NH * T16], bf16, isOutput=False)
    if T8:
        x8 = nc.declare_dram_parameter("x8", [T8 * P, D], fp8, isOutput=False)
        sel8 = nc.declare_dram_parameter(
            "sel8", [P, T2 * 2 * SEL_PAD], fp8, isOutput=False
        )
    scale = nc.declare_dram_parameter("scale", [BPC, 1], f32, isOutput=False)
    out = nc.declare_dram_parameter("out", [BPC, D], f32, isOutput=True)

    n_acc = T16 + T2  # accumulation steps (plain groups + super-groups)
    n_w4 = T16 + (T2 - K2)  # steps whose matmuls write all 4 PSUM rows

    tiles16 = _split16(T16 * P) if T16 else []
    tiles8 = _split8(T8 * P) if T8 else []

    with tile.TileContext(nc) as tc:
        with (
            tc.tile_pool(name="xin", bufs=5) as xpool,
            tc.tile_pool(name="acc", bufs=1, space="PSUM") as psum_pool,
            tc.tile_pool(name="aux", bufs=1) as aux,
        ):
            # Dispatch the FIRST x tile before the tiny sel/scale loads so
            # the big stream starts ~1.5us earlier (dispatches are serial on
            # the Sync engine; the selector transfers are <0.2us).
            first16_xt = first8_xt = None
            if T16:
                r0 = tiles16[0]
                first16_xt = xpool.tile([P, (r0 // P) * D], bf16, tag="xt16")
                nc.sync.dma_start(
                    first16_xt[:],
                    x16.ap()[0:r0, :].rearrange("(p a) d -> p (a d)", p=P),
                )
            elif T8:
                r0 = tiles8[0]
                first8_xt = xpool.tile([P, r0 // 256, 2, D], fp8, tag="xt8")
                nc.sync.dma_start(
                    first8_xt[:],
                    x8.ap()[0:r0, :].rearrange("(p a) d -> p (a d)", p=P),
                )

            if T16:
                sel16_sb = aux.tile([P, NH * T16], bf16)
                nc.sync.dma_start(sel16_sb[:], sel16.ap())
            if T8:
                sel8_sb = aux.tile([P, T2, 2, SEL_PAD], fp8)
                nc.sync.dma_start(sel8_sb[:], sel8.ap())
            scale_sb = aux.tile([BPC, 1], f32)
            nc.sync.dma_start(scale_sb[:], scale.ap())

            ps = psum_pool.tile([BPC, D], f32)
            out_sb = aux.tile([BPC, D], f32)
            a_idx = 0

            def emit_epilogue_123():
                # Rows 1-3 are final once the last full-width matmul
                # retires; scale + store them while the pure slot-0 tail
                # still streams. DVE partition access must start at
                # partition 0, so the op covers [0:4] and row 0 (still
                # accumulating; value discarded) is simply not DMA'd.
                nc.vector.tensor_scalar_mul(
                    out_sb[0:BPC, :], ps[0:BPC, :], scale_sb[0:BPC, 0:1]
                )
                nc.sync.dma_start(out.ap()[1:BPC, :], out_sb[1:BPC, :])

            # bf16 region: plain matmuls over 128-row groups.
            if T16:
                row_off = 0
                t_idx = 0
                for ti, rows in enumerate(tiles16):
                    rpp = rows // P
                    if ti == 0:
                        xt = first16_xt
                    else:
                        xt = xpool.tile([P, rpp * D], bf16, tag="xt16")
                        nc.sync.dma_start(
                            xt[:],
                            x16.ap()[row_off : row_off + rows, :].rearrange(
                                "(p a) d -> p (a d)", p=P
                            ),
                        )
                    row_off += rows
                    for r in range(rpp):
                        w = sel16_sb[:, NH * t_idx : NH * (t_idx + 1)]
                        for h in range(D // 512):
                            c0 = r * D + h * 512
                            nc.tensor.matmul(
                                ps[0:BPC, h * 512 : (h + 1) * 512],
                                w,
                                xt[:, c0 : c0 + 512],
                                start=(a_idx == 0),
                                stop=(a_idx == n_acc - 1),
                            )
                        t_idx += 1
                        a_idx += 1
                        if a_idx == n_w4:
                            emit_epilogue_123()
                assert t_idx == T16

            # fp8 region: DoubleRow matmuls; trailing K2 super-groups are
            # width-1 (PSUM row 3 only).
            if T8:
                row_off = 0
                t2_idx = 0
                for ti, rows in enumerate(tiles8):
                    g2 = rows // 256
                    if ti == 0 and first8_xt is not None:
                        xt = first8_xt
                    else:
                        xt = xpool.tile([P, g2, 2, D], fp8, tag="xt8")
                        nc.sync.dma_start(
                            xt[:],
                            x8.ap()[row_off : row_off + rows, :].rearrange(
                                "(p a) d -> p (a d)", p=P
                            ),
                        )
                    row_off += rows
                    for g in range(g2):
                        # Past n_w4 steps the selector columns for slots
                        # 1-3 are all zero (pure slot-0 tail), so these
                        # matmuls only add zeros to rows 1-3. The rows-1-3
                        # epilogue is emitted at the n_w4 boundary; its WAR
                        # dependency orders the harmless +0 writes after the
                        # read.
                        for h in range(D // 512):
                            nc.tensor.matmul(
                                ps[0:BPC, h * 512 : (h + 1) * 512],
                                sel8_sb[:, t2_idx, :, 0:NH],
                                xt[:, g, :, h * 512 : (h + 1) * 512],
                                start=(a_idx == 0),
                                stop=(a_idx == n_acc - 1),
                                perf_mode=mybir.MatmulPerfMode.DoubleRow,
                            )
                        t2_idx += 1
                        a_idx += 1
                        if a_idx == n_w4:
                            emit_epilogue_123()
                assert t2_idx == T2
            assert a_idx == n_acc

            # Row 0 finishes with the last matmul: small copy + 4 KB DMA.
            nc.vector.tensor_scalar_mul(
                out_sb[0:1, :], ps[0:1, :], scale_sb[0:1, 0:1]
            )
            nc.sync.dma_start(out.ap()[0:1, :], out_sb[0:1, :])

    nc.compile()
    return nc


def _pack_bins(lengths):
    """Assign samples to cores (BPC each), minimizing the padded stream cost
    (T8 + 2*T16 blocks, then total groups, then max rows) via LPT seed +
    randomized swaps."""
    nrows = np.maximum(1, lengths).astype(np.int64)
    is8 = nrows >= THRESH

    def cost(bins_):
        r8 = [sum(int(nrows[i]) for i in b if is8[i]) for b in bins_]
        r16 = [sum(int(nrows[i]) for i in b if not is8[i]) for b in bins_]
        T8 = max(-(-r // P) for r in r8)
        T16 = max(-(-r // P) for r in r16)
        return (T8 + 2 * T16, T8 + T16, max(a + b for a, b in zip(r8, r16)))

    bins = [[] for _ in range(N_CORES)]
    tot = [0] * N_CORES
    for i in np.argsort(-nrows, kind="stable"):
        c = min(
            (c for c in range(N_CORES) if len(bins[c]) < BPC),
            key=lambda c: (tot[c], len(bins[c])),
        )
        bins[c].append(int(i))
        tot[c] += int(nrows[i])

    best = cost(bins)
    rng = np.random.RandomState(0)
    for _ in range(4000):
        c1, c2 = rng.randint(0, N_CORES, 2)
        if c1 == c2:
            continue
        a, b = rng.randint(0, BPC, 2)
        bins[c1][a], bins[c2][b] = bins[c2][b], bins[c1][a]
        cand = cost(bins)
        if cand <= best:
            best = cand
        else:
            bins[c1][a], bins[c2][b] = bins[c2][b], bins[c1][a]

    # Pin each core's longest fp8 sample to slot 0 (it is packed at the END
    # of the fp8 stream, enabling the pure-tail epilogue overlap; row 0 is
    # the only legal base partition for the final DVE op).
    for b in bins:
        i8 = [i for i in b if is8[i]]
        if i8:
            j = max(i8, key=lambda i: nrows[i])
            k = b.index(j)
            b[k], b[0] = b[0], b[k]
    return bins


def kernel(**inputs) -> np.ndarray:
    global LAST_RESULTS
    x = np.asarray(inputs["encoded_batch"])
    if x.dtype != np.float32:
        x = x.astype(np.float32)
    lengths = np.asarray(inputs["text_lengths"]).astype(np.int64)
    assert x.shape == (B, S, D), x.shape

    nrows = np.maximum(1, lengths).astype(np.int64)
    is8 = nrows >= THRESH
    bins = _pack_bins(lengths)
    r8 = [sum(int(nrows[i]) for i in b if is8[i]) for b in bins]
    r16 = [sum(int(nrows[i]) for i in b if not is8[i]) for b in bins]
    T8 = max(-(-r // P) for r in r8)
    T8 += T8 % 2  # DoubleRow super-groups need an even block count
    T16 = max(-(-r // P) for r in r16)

    # Pure tail: trailing fp8 tiles whose rows all belong to slot 3 (or
    # padding) on EVERY core. A super-group's rows stride across its whole
    # tile, so purity is a per-tile property.
    tiles8 = _split8(T8 * P) if T8 else []
    off0 = []
    for c in range(N_CORES):
        s0 = bins[c][0]
        len0 = int(nrows[s0]) if is8[s0] else 0
        off0.append(r8[c] - len0)
    off_max = max(off0) if off0 else 0
    K2 = 0
    base = T8 * P
    for rows in reversed(tiles8):
        base -= rows
        if base >= off_max:
            K2 += rows // 256
        else:
            break
    if T8 == 0:
        K2 = 0

    key = (T16, T8, K2)
    if key not in _CACHE:
        _CACHE[key] = _build(T16, T8, K2)
    nc = _CACHE[key]

    inv = (np.float64(1.0) / lengths.astype(np.float64)).astype(np.float32)
    pidx = np.arange(P)

    def pack_stream(samples, T, np_dt):
        xp = np.zeros((T * P, D), dtype=np_dt)
        row_slot = np.full(T * P, -1, dtype=np.int64)
        off = 0
        for m, i in samples:
            nr = int(nrows[i])
            xp[off : off + nr] = x[i, :nr].astype(np_dt)
            row_slot[off : off + nr] = m
            off += nr
        return xp, row_slot

    def sel_plain(row_slot, T, tiles):
        selc = np.zeros((P, NH * T), dtype=BF16)
        t = 0
        base = 0
        for rows_ in tiles:
            rpp = rows_ // P
            for r in range(rpp):
                rs = row_slot[base + pidx * rpp + r]
                valid = rs >= 0
                selc[pidx[valid], NH * t + rs[valid]] = 1.0
                t += 1
            base += rows_
        assert t == T
        return selc

    def sel_double(row_slot, T2, tiles):
        selc = np.zeros((P, T2, 2, 16), dtype=FP8)
        t = 0
        base = 0
        for rows_ in tiles:
            g2 = rows_ // 256
            q = 2 * g2
            for g in range(g2):
                for j in range(2):
                    rs = row_slot[base + pidx * q + 2 * g + j]
                    valid = rs >= 0
                    selc[pidx[valid], t, j, rs[valid]] = 1.0
                t += 1
            base += rows_
        assert t == T2
        return selc.reshape(P, T2 * 2 * 16)

    tiles16 = _split16(T16 * P) if T16 else []
    in_maps = []
    for c in range(N_CORES):
        m8 = [(m, i) for m, i in enumerate(bins[c]) if is8[i]]
        m8.sort(key=lambda t: (t[0] == 0, t[0]))
        m8 = [t for t in m8 if t[0] != 0] + [t for t in m8 if t[0] == 0]
        m16 = [(m, i) for m, i in enumerate(bins[c]) if not is8[i]]
        im = {"scale": inv[bins[c]].reshape(BPC, 1)}
        if T16:
            im["x16"], slot16 = pack_stream(m16, T16, BF16)
            im["sel16"] = sel_plain(slot16, T16, tiles16)
        if T8:
            im["x8"], slot8 = pack_stream(m8, T8, FP8)
            im["sel8"] = sel_double(slot8, T8 // 2, tiles8)
        in_maps.append(im)

    res = run_bass_kernel_spmd(nc, in_maps, list(range(N_CORES)))
    LAST_RESULTS = res

    full = np.empty((B, D * NH), dtype=np.float32)
    for c in range(N_CORES):
        mean_c = res.results[c]["out"]  # [BPC, D] f32
        full[bins[c]] = np.repeat(mean_c, NH, axis=-1)
    return full
